# revision 1
# baseline (speedup 1.0000x reference)
# Multi-head self-attention (B=2, S=4096, D=512, H=8) on 8 NeuronCores.
#
# Sharding: core c -> batch b = c//4, head-pair hp = c%4 (heads 2hp, 2hp+1,
# i.e. channels [128*hp, 128*hp+128) of the QKV projection space).
# Host pre-slices/transposes weights + x per core (cast fp16 for the PE);
# device does all matmuls (QKV projections, flash-attention with fused
# softmax, W_O row-slice projection); host sums the 4 per-core W_O partials
# per batch (the "all-reduce") and transposes back.
#
# Per-core device kernel (matmul operands fp16, accumulation fp32 PSUM):
#   qtz_h/ktz_h [128, 4096]: Q^T/K^T per head, dk on a 64-partition band,
#     zero elsewhere -> every attention matmul is full-K (128,128) mode.
#   scoresT[kpos, q] = K Q^T chunkwise -> ACT exp(x/8) straight from PSUM
#   AV with a ones-column appended to V -> denominator for free
#   softmax division off the critical path (DVE recip + gpsimd broadcast).
# All pools stay open for the whole kernel; PSUM slots are shared between
# phases via tags (8 banks total) so phases overlap with per-slot WAR deps
# instead of pool-close barriers.

import numpy as np

B, S, D, H, DK = 2, 4096, 512, 8, 64
P = 128          # partition tile
NQ = 512         # matmul moving free dim (one fp32 PSUM bank)
QCH = 1024       # q-chunk (2 x NQ) => one [128,1024] exp per kpos-chunk
NKC = S // P     # kpos chunks (32)
NST = S // NQ    # s-tiles of 512 (8)
NDC = D // P     # d chunks (4)
NQC = S // QCH   # q chunks (4)

TRACE = False            # test.py sets True to get exec_time_ns + perfetto
TMPDIR = None            # optional trace output dir
LAST_RESULTS = None      # BassKernelResults of the last run (for test.py)

_CACHE = {}


def _build_nc():
    import concourse.bass as bass  # noqa: F401
    import concourse.mybir as mybir
    import concourse.tile as tile
    from concourse import bacc
    from concourse.masks import make_identity

    f32 = mybir.dt.float32
    f16 = mybir.dt.float16
    Act = mybir.ActivationFunctionType

    nc = bacc.Bacc("TRN2", target_bir_lowering=False, debug=False, num_devices=8)

    xT = nc.dram_tensor("xT", [D, S], f16, kind="ExternalInput")
    wqT = nc.dram_tensor("wqT", [D, P], f16, kind="ExternalInput")
    wkT = nc.dram_tensor("wkT", [D, P], f16, kind="ExternalInput")
    wvT = nc.dram_tensor("wvT", [D, P], f16, kind="ExternalInput")
    woT0 = nc.dram_tensor("woT0", [DK, D], f16, kind="ExternalInput")
    woT1 = nc.dram_tensor("woT1", [DK, D], f16, kind="ExternalInput")
    yT = nc.dram_tensor("yT", [D, S], f32, kind="ExternalOutput")

    with tile.TileContext(nc) as tc:
        with (
            tc.tile_pool(name="sb", bufs=1) as sb,
            tc.tile_pool(name="ps", bufs=1, space="PSUM") as psp,
        ):
            # PSUM budget (8 banks total, slots shared across phases by tag):
            #   sc0, sc1: [128,1024] -> 2 banks each (scores / exp staging)
            #   av00..av11: [128,512] -> 1 bank each (AV accum; also used by
            #   the QKV-projection psum tiles and the W_O psum tiles)
            def av_ps(i, shape):
                return psp.tile(shape, f32, tag=f"av{i % 4}", name=f"avps{i}")

            # ---- persistent operand tiles -----------------------------------
            qtz = [sb.tile([P, S], f16, tag=f"qtz{h}", name=f"qtz{h}")
                   for h in range(2)]
            ktz = [sb.tile([P, S], f16, tag=f"ktz{h}", name=f"ktz{h}")
                   for h in range(2)]
            vb = [sb.tile([P, NKC * (DK + 1)], f16, tag=f"vb{h}", name=f"vb{h}")
                  for h in range(2)]
            outtz = [sb.tile([P, S], f16, tag=f"outtz{h}", name=f"outtz{h}")
                     for h in range(2)]
            wosz = [sb.tile([P, D], f16, tag=f"wosz{h}", name=f"wosz{h}")
                    for h in range(2)]

            # zero bands + ones-fill on the (otherwise idle) gpsimd engine;
            # bands first: the first scores matmul needs them, vb is needed
            # slightly later by the first AV matmul
            nc.gpsimd.memset(qtz[0][DK:P, :], 0.0)
            nc.gpsimd.memset(ktz[0][DK:P, :], 0.0)
            nc.gpsimd.memset(qtz[1][0:DK, :], 0.0)
            nc.gpsimd.memset(ktz[1][0:DK, :], 0.0)
            nc.gpsimd.memset(vb[0][:, :], 1.0)
            nc.gpsimd.memset(vb[1][:, :], 1.0)
            nc.gpsimd.memset(outtz[0][DK:P, :], 0.0)
            nc.gpsimd.memset(outtz[1][DK:P, :], 0.0)
            nc.gpsimd.memset(wosz[0][DK:P, :], 0.0)
            nc.gpsimd.memset(wosz[1][DK:P, :], 0.0)

            # ---- phase 1: load x + weights, QKV projections, build V -------
            xts = [sb.tile([P, S], f16, tag=f"xt{dc}", name=f"xt{dc}")
                   for dc in range(NDC)]
            wsb = {}
            for name, dram in (("v", wvT), ("k", wkT), ("q", wqT)):
                w = sb.tile([P, NDC * P], f16, tag=f"w{name}", name=f"w{name}")
                for dc in range(NDC):
                    nc.sync.dma_start(
                        w[:, dc * P:(dc + 1) * P], dram[dc * P:(dc + 1) * P, :]
                    )
                wsb[name] = w
            nc.sync.dma_start(wosz[0][0:DK, :], woT0[:, :])
            nc.sync.dma_start(wosz[1][0:DK, :], woT1[:, :])
            for blk in range(8):
                sl = slice(blk * NQ, (blk + 1) * NQ)
                for dc in range(NDC):
                    nc.sync.dma_start(xts[dc][:, sl], xT[dc * P:(dc + 1) * P, sl])

            vt = sb.tile([P, S], f32, tag="vt")
            ident = sb.tile([P, P], f32, tag="ident")
            make_identity(nc, ident[:, :])

            psn = 0
            for name in ("v", "k", "q"):
                w = wsb[name]
                for st in range(NST):
                    ps = av_ps(psn, [P, NQ])
                    psn += 1
                    for dc in range(NDC):
                        nc.tensor.matmul(
                            ps[:, :],
                            w[:, dc * P:(dc + 1) * P],
                            xts[dc][:, st * NQ:(st + 1) * NQ],
                            start=(dc == 0),
                            stop=(dc == NDC - 1),
                        )
                    sl = slice(st * NQ, (st + 1) * NQ)
                    if name == "v":
                        nc.vector.tensor_copy(vt[:, sl], ps[:, :])
                        # transpose this V window into per-head V chunks
                        # [kpos, dk] (+ones col at 64) right away
                        for ch in range(4 * st, 4 * st + 4):
                            tp = av_ps(psn, [P, P])
                            psn += 1
                            nc.tensor.transpose(
                                tp[:, :], vt[:, ch * P:(ch + 1) * P], ident[:, :]
                            )
                            c0 = ch * (DK + 1)
                            nc.scalar.copy(vb[0][:, c0:c0 + DK], tp[:, 0:DK])
                            nc.vector.tensor_copy(
                                vb[1][:, c0:c0 + DK], tp[:, DK:P]
                            )
                    elif name == "k":
                        nc.scalar.copy(ktz[0][0:DK, sl], ps[0:DK, :])
                        nc.scalar.copy(ktz[1][DK:P, sl], ps[DK:P, :])
                    else:
                        nc.vector.tensor_copy(qtz[0][0:DK, sl], ps[0:DK, :])
                        nc.vector.tensor_copy(qtz[1][DK:P, sl], ps[DK:P, :])

            # ---- phase 2: flash attention -----------------------------------
            def emit_normalize(qc, av):
                # evacuate av psum fast (frees the bank), then the softmax
                # division off the critical path in SBUF
                for h in range(2):
                    for sub in range(2):
                        a = av[h, sub]
                        raw = sb.tile([DK + 1, NQ], f32, tag=f"raw{h}{sub}",
                                      name=f"raw{h}{sub}", bufs=2)
                        nc.vector.tensor_copy(raw[:, :], a[0:DK + 1, :])
                        dn0 = sb.tile([P, NQ], f32, tag="dn0", bufs=2)
                        nc.sync.dma_start(dn0[0:1, :], raw[DK:DK + 1, :])
                        rc = sb.tile([P, NQ], f32, tag="rc", bufs=2)
                        nc.vector.reciprocal_approx_fast(rc[0:1, :], dn0[0:1, :])
                        rcb = sb.tile([DK, NQ], f32, tag="rcb", bufs=2)
                        nc.gpsimd.partition_broadcast(
                            rcb[:, :], rc[0:1, :], channels=DK
                        )
                        q0 = qc * QCH + sub * NQ
                        nc.vector.tensor_mul(
                            outtz[h][0:DK, q0:q0 + NQ], raw[0:DK, :], rcb[:, :]
                        )

            pending = None
            for qc in range(NQC):
                av = {}
                for h in range(2):
                    for sub in range(2):
                        av[h, sub] = av_ps(psn, [P, NQ])
                        psn += 1
                for k in range(NKC):
                    if k == 3 and pending is not None:
                        emit_normalize(*pending)
                        pending = None
                    scps = []
                    for h in range(2):
                        scp = psp.tile([P, QCH], f32, tag=f"sc{h}",
                                       name=f"sc{h}")
                        scps.append(scp)
                        for sub in range(2):
                            q0 = qc * QCH + sub * NQ
                            nc.tensor.matmul(
                                scp[:, sub * NQ:(sub + 1) * NQ],
                                ktz[h][:, k * P:(k + 1) * P],
                                qtz[h][:, q0:q0 + NQ],
                                start=True,
                                stop=True,
                            )
                    exs = []
                    for h in range(2):
                        ex = sb.tile([P, QCH], f16, tag=f"ex{h}", name=f"ex{h}",
                                     bufs=3)
                        exs.append(ex)
                        nc.scalar.activation(
                            ex[:, :], scps[h][:, :], Act.Exp, scale=0.125
                        )
                    c0 = k * (DK + 1)
                    for h in range(2):
                        for sub in range(2):
                            nc.tensor.matmul(
                                av[h, sub][0:DK + 1, :],
                                vb[h][:, c0:c0 + DK + 1],
                                exs[h][:, sub * NQ:(sub + 1) * NQ],
                                start=(k == 0),
                                stop=(k == NKC - 1),
                            )
                pending = (qc, av)
            emit_normalize(*pending)

            # ---- phase 3: W_O row-slice projection --------------------------
            # yT[e, s] = sum_h wosz_h.T @ outtz_h (K padded to 128 w/ zeros);
            # psum evacuations on the (now idle) scalar engine
            for st in range(NST):
                for ec in range(NDC):
                    yp = av_ps(psn, [P, NQ])
                    psn += 1
                    for h in range(2):
                        nc.tensor.matmul(
                            yp[:, :],
                            wosz[h][:, ec * P:(ec + 1) * P],
                            outtz[h][:, st * NQ:(st + 1) * NQ],
                            start=(h == 0),
                            stop=(h == 1),
                        )
                    ys = sb.tile([P, NQ], f32, tag="ys", bufs=4)
                    nc.scalar.copy(ys[:, :], yp[:, :])
                    nc.sync.dma_start(
                        yT[ec * P:(ec + 1) * P, st * NQ:(st + 1) * NQ],
                        ys[:, :],
                    )

    nc.compile()
    return nc


def kernel(x, wq, wk, wv, wo):
    global LAST_RESULTS
    from concourse.bass_utils import run_bass_kernel_spmd

    if "nc" not in _CACHE:
        _CACHE["nc"] = _build_nc()
    nc = _CACHE["nc"]

    x = np.asarray(x, dtype=np.float32)
    wq = np.asarray(wq, dtype=np.float32)
    wk = np.asarray(wk, dtype=np.float32)
    wv = np.asarray(wv, dtype=np.float32)
    wo = np.asarray(wo, dtype=np.float32)

    in_maps = []
    for c in range(8):
        b, hp = divmod(c, 4)
        e0 = hp * P
        in_maps.append({
            "xT": np.ascontiguousarray(x[b].T.astype(np.float16)),
            "wqT": np.ascontiguousarray(wq[e0:e0 + P].T.astype(np.float16)),
            "wkT": np.ascontiguousarray(wk[e0:e0 + P].T.astype(np.float16)),
            "wvT": np.ascontiguousarray(wv[e0:e0 + P].T.astype(np.float16)),
            "woT0": np.ascontiguousarray(wo[:, e0:e0 + DK].T.astype(np.float16)),
            "woT1": np.ascontiguousarray(wo[:, e0 + DK:e0 + P].T.astype(np.float16)),
        })

    res = run_bass_kernel_spmd(
        nc, in_maps, core_ids=list(range(8)), trace=TRACE, tmpdir=TMPDIR
    )
    LAST_RESULTS = res

    y = np.zeros((B, S, D), dtype=np.float32)
    for c in range(8):
        y[c // 4] += res.results[c]["yT"].T
    return y

